# revision 9
# baseline (speedup 1.0000x reference)
"""Sparse cross-attention kernel for Trainium2 (8 NeuronCores, SPMD).

Problem: nn_CrossAttn (NP=1024 queries, MNP=4096 keys, BS=4, DIM=512,
NH=8 heads, dh=64, TOPK=32 sparse mask shared across heads).

Sharding: core = (batch b, head-group hg).  4 batches x 2 head-groups of 4
heads each.  Each core computes its batch's attention for its 4 heads and
writes a (256 ch, 1024 q) transposed output block; the host reassembles.

v2 design (phase-batched, all PE matmuls in 128x128 mode - no tiling-mode
switches):
  - scores use K=128 with zero-padded per-head qhT (qhTz0/qhTz1) so the
    score and AV matmuls share one PE tiling mode and pipeline cleanly.
  - per (qt, hp) "group" of 32 key chunks: slot i emits scores+exp+mask for
    group i interleaved with AV matmuls of group i-1 (numerm ring of 32).
  - exp split: ACT table-exp for 3/4 chunks; DVE Schraudolph bit-trick exp
    (int32 tensor_scalar + bitcast) for kc%4==3.
  - mask multiply split: GpSimd for kc%8 in {1,2,5} (bf16 chunks), DVE else.
  - V projection has no bias matmuls: b3 passes through softmax (weights sum
    to 1) and is added at the tail; 'ones' denominator columns are memset.
"""

import numpy as np
import ml_dtypes

import concourse.bass as bass
import concourse.mybir as mybir
import concourse.tile as tile
from concourse.bass_utils import run_bass_kernel_spmd

BF16 = mybir.dt.bfloat16
F32 = mybir.dt.float32
I32 = mybir.dt.int32
AF = mybir.ActivationFunctionType
ALU = mybir.AluOpType

NH = 8
DIM = 512
NP = 1024
MNP = 4096
BS = 4
DH = 64
N_CORES = 8
HG_CH = 256          # channels per head-group (4 heads x 64)
NKC = MNP // 128     # 32 key chunks

# Schraudolph fast-exp constants (tuned C for scores in [-2.5, 2.5])
SCH_A = float(np.float32(2 ** 23 / np.log(2.0)))
SCH_B = float(np.float32(127 * 2 ** 23 - 366000))

GROUPS = [(0, 0), (0, 1), (1, 0), (1, 1)]  # (qt, hp)


def _is_dve_exp(kc):
    return kc % 8 == 3


def _is_gp_mask(kc):
    return kc % 8 in (1, 2, 5)


# options test.py can flip
run_opts = {"trace": False, "trace_kwargs": {}}
_last_results = {}


def _split_multi_waits(nc):
    """This container's walrus encodes only ONE sync-wait per TPB instruction
    (newer Tile emits several).  Split extras onto preceding NOPs."""
    eng_ok = {
        mybir.EngineType.PE,
        mybir.EngineType.Activation,
        mybir.EngineType.DVE,
        mybir.EngineType.Pool,
        mybir.EngineType.SP,
    }
    for fn in nc.m.functions:
        for blk in fn.blocks:
            insts = blk.instructions
            out = []
            changed = False
            for inst in insts:
                si = inst.sync_info
                if (
                    si is not None
                    and si.on_wait
                    and len(si.on_wait) > 1
                    and inst.engine in eng_ok
                ):
                    waits = list(si.on_wait)
                    for j, w in enumerate(waits[:-1]):
                        out.append(
                            mybir.InstNoOp(
                                name=f"{inst.name}-w{j}",
                                engine=inst.engine,
                                ins=[],
                                outs=[],
                                sync_info=mybir.SyncInfo(on_wait=[w], on_update=[]),
                            )
                        )
                    inst.sync_info = mybir.SyncInfo(
                        on_wait=[waits[-1]], on_update=list(si.on_update)
                    )
                    changed = True
                out.append(inst)
            if changed:
                blk.instructions = out


def _build_nc() -> bass.Bass:
    nc = bass.Bass()

    qt_d = nc.dram_tensor("qt", [128, 4, NP], BF16, kind="ExternalInput")
    kt_d = nc.dram_tensor("kt", [128, 4, MNP], BF16, kind="ExternalInput")
    vt_d = nc.dram_tensor("vt", [128, 4, MNP], BF16, kind="ExternalInput")
    w1t_d = nc.dram_tensor("w1t", [128, 4, HG_CH], BF16, kind="ExternalInput")
    w2t_d = nc.dram_tensor("w2t", [128, 4, HG_CH], BF16, kind="ExternalInput")
    w3t_d = nc.dram_tensor("w3t", [128, 4, 260], BF16, kind="ExternalInput")
    b1c_d = nc.dram_tensor("b1c", [128, 2], F32, kind="ExternalInput")
    b2c_d = nc.dram_tensor("b2c", [128, 2], F32, kind="ExternalInput")
    b3c_d = nc.dram_tensor("b3c", [128, 2], F32, kind="ExternalInput")
    mask_d = nc.dram_tensor("maskt", [128, NKC, NP], BF16, kind="ExternalInput")
    ones64_d = nc.dram_tensor("ones64", [1, 64], F32, kind="ExternalInput")
    out_d = nc.dram_tensor("outt", [2, 128, 2, 512], F32, kind="ExternalOutput")

    with tile.TileContext(nc) as tc:
        with (
            tc.tile_pool(name="const", bufs=1) as const,
            tc.tile_pool(name="big", bufs=1) as big,
            tc.tile_pool(name="kio", bufs=2) as kio,
            tc.tile_pool(name="vio", bufs=2) as vio,
            tc.tile_pool(name="nbp", bufs=2) as nbp,
            tc.tile_pool(name="nip", bufs=1) as nip,
            tc.tile_pool(name="nmp", bufs=32) as nmp,
            tc.tile_pool(name="work", bufs=1) as work,
            tc.tile_pool(name="fin", bufs=2) as fin,
            tc.tile_pool(name="ps_s", bufs=2, space="PSUM") as ps_s,
            tc.tile_pool(name="ps_o", bufs=1, space="PSUM") as ps_o,
            tc.tile_pool(name="ps_p", bufs=2, space="PSUM") as ps_p,
        ):
            # ---- constants / weights ----
            w1t = const.tile([128, 4, HG_CH], BF16, tag="w1t")
            w2t = const.tile([128, 4, HG_CH], BF16, tag="w2t")
            w3t = const.tile([128, 4, 260], BF16, tag="w3t")
            b1c = const.tile([128, 2], F32, tag="b1c")
            b2c = const.tile([128, 2], F32, tag="b2c")
            b3c = const.tile([128, 2], F32, tag="b3c")
            ones64 = const.tile([1, 64], F32, tag="ones64")
            for sb, dr in (
                (w1t, w1t_d), (w2t, w2t_d), (w3t, w3t_d),
                (b1c, b1c_d), (b2c, b2c_d), (b3c, b3c_d), (ones64, ones64_d),
            ):
                nc.sync.dma_start(out=sb[:], in_=dr[:])

            # ---- big resident tensors ----
            maskt = big.tile([128, NKC, NP], BF16, tag="maskt")
            for i in range(8):
                nc.sync.dma_start(
                    out=maskt[:, 4 * i:4 * (i + 1), :],
                    in_=mask_d[:, 4 * i:4 * (i + 1), :],
                )

            qhTz0 = big.tile([128, 2, NP], BF16, tag="qhTz0")
            qhTz1 = big.tile([128, 2, NP], BF16, tag="qhTz1")
            khT = big.tile([128, 2, MNP], BF16, tag="khT")
            vh = big.tile([128, NKC, 260], BF16, tag="vh")

            # zero halves of padded q, ones-columns of vh
            nc.gpsimd.memset(qhTz0[64:128, :, :], 0.0)
            nc.gpsimd.memset(qhTz1[0:64, :, :], 0.0)
            vh_ones = vh[:].rearrange("p k (h c) -> p k h c", h=4)[:, :, :, 64:65]
            nc.gpsimd.memset(vh_ones, 1.0)

            # ---- q projection (zero-padded per head halves) ----
            def proj_q_chunk(nq):
                qts = kio.tile([128, 4, 512], BF16, tag="kts", name=f"qts{nq}")
                nc.sync.dma_start(out=qts[:], in_=qt_d[:, :, nq * 512:(nq + 1) * 512])
                for pair in range(2):
                    pt = ps_p.tile([128, 512], F32, tag="pp", name=f"qp{nq}{pair}")
                    for c in range(4):
                        nc.tensor.matmul(
                            pt[:],
                            lhsT=w1t[:, c, pair * 128:(pair + 1) * 128],
                            rhs=qts[:, c, :],
                            start=(c == 0),
                            stop=(c == 3),
                        )
                    nc.vector.tensor_tensor(
                        out=qhTz0[0:64, pair, nq * 512:(nq + 1) * 512],
                        in0=pt[0:64, :],
                        in1=b1c[0:64, pair:pair + 1].to_broadcast((64, 512)),
                        op=ALU.add,
                    )
                    nc.vector.tensor_tensor(
                        out=qhTz1[64:128, pair, nq * 512:(nq + 1) * 512],
                        in0=pt[64:128, :],
                        in1=b1c[64:128, pair:pair + 1].to_broadcast((64, 512)),
                        op=ALU.add,
                    )

            def proj_k_chunk(nq):
                kts = kio.tile([128, 4, 512], BF16, tag="kts", name=f"kts{nq}")
                nc.sync.dma_start(out=kts[:], in_=kt_d[:, :, nq * 512:(nq + 1) * 512])
                for pair in range(2):
                    pt = ps_p.tile([128, 512], F32, tag="pp", name=f"kp{nq}{pair}")
                    for c in range(4):
                        nc.tensor.matmul(
                            pt[:],
                            lhsT=w2t[:, c, pair * 128:(pair + 1) * 128],
                            rhs=kts[:, c, :],
                            start=(c == 0),
                            stop=(c == 3),
                        )
                    nc.vector.tensor_tensor(
                        out=khT[:, pair, nq * 512:(nq + 1) * 512],
                        in0=pt[:],
                        in1=b2c[:, pair:pair + 1].to_broadcast((128, 512)),
                        op=ALU.add,
                    )

            def proj_v_chunk(kc):
                vts = vio.tile([128, 4, 128], BF16, tag="vts", name=f"vts{kc}")
                nc.sync.dma_start(out=vts[:], in_=vt_d[:, :, kc * 128:(kc + 1) * 128])
                pt = ps_p.tile([128, 260], F32, tag="pp", name=f"vp{kc}")
                for c in range(4):
                    nc.tensor.matmul(
                        pt[:, 0:260],
                        lhsT=vts[:, c, :],
                        rhs=w3t[:, c, :],
                        start=(c == 0),
                        stop=(c == 3),
                    )
                # copy only the 4x64 data columns; ones columns stay memset
                nc.scalar.copy(
                    out=vh[:, kc, :].rearrange("p (h c) -> p h c", h=4)[:, :, 0:64],
                    in_=pt[:, 0:260].rearrange("p (h c) -> p h c", h=4)[:, :, 0:64],
                )

            # ---- attention pieces ----
            def score_chunk(qt, hp, kc):
                s_ps = ps_s.tile([128, 1024], F32, tag="s", name=f"s{qt}{hp}{kc}")
                for h, qz in ((0, qhTz0), (1, qhTz1)):
                    nc.tensor.matmul(
                        s_ps[:, h * 512:(h + 1) * 512],
                        lhsT=khT[:, hp, kc * 128:(kc + 1) * 128],
                        rhs=qz[:, hp, qt * 512:(qt + 1) * 512],
                        start=True,
                        stop=True,
                    )
                return s_ps

            def exp_chunk(qt, hp, kc, s_ps):
                if _is_dve_exp(kc):
                    ni = nip.tile([128, 1024], I32, tag="ni", name=f"ni{qt}{hp}{kc}")
                    nc.vector.tensor_scalar(
                        out=ni[:], in0=s_ps[:],
                        scalar1=SCH_A, scalar2=SCH_B,
                        op0=ALU.mult, op1=ALU.add,
                    )
                    return ni
                nb = nbp.tile([128, 1024], BF16, tag="nb", name=f"nb{qt}{hp}{kc}")
                nc.scalar.activation(nb[:], s_ps[:], AF.Exp)
                return nb

            def mask_chunk(qt, hp, kc, nsrc):
                nm = nmp.tile([128, 1024], BF16, tag="nm", name=f"nm{qt}{hp}{kc}")
                eng = nc.gpsimd if (_is_gp_mask(kc) and not _is_dve_exp(kc)) else nc.vector
                for h in range(2):
                    in0 = nsrc[:, h * 512:(h + 1) * 512]
                    if _is_dve_exp(kc):
                        in0 = in0.bitcast(F32)
                    eng.tensor_tensor(
                        out=nm[:, h * 512:(h + 1) * 512],
                        in0=in0,
                        in1=maskt[:, kc, qt * 512:(qt + 1) * 512],
                        op=ALU.mult,
                    )
                return nm

            def av_chunk(hp, kc, nm, o_ps):
                for h in range(2):
                    ch = (2 * hp + h) * 65
                    nc.tensor.matmul(
                        o_ps[h][:],
                        lhsT=vh[:, kc, ch:ch + 65],
                        rhs=nm[:, h * 512:(h + 1) * 512],
                        start=(kc == 0),
                        stop=(kc == NKC - 1),
                    )

            def attn_tail(qt, hp, o_ps):
                # free o_ps quickly: copy [65,512] (row 64 = denominator)
                osb = [
                    fin.tile([65, 512], F32, tag=f"osb{h}", bufs=1, name=f"osb{qt}{hp}{h}")
                    for h in range(2)
                ]
                for h in range(2):
                    nc.vector.tensor_copy(out=osb[h][:], in_=o_ps[h][:])
                # transpose den rows to partitions via DMA, reciprocal, back
                drow = work.tile([128, 8], F32, tag="drow", name=f"drow{qt}{hp}")
                for h in range(2):
                    nc.sync.dma_start(
                        out=drow[:, 4 * h:4 * (h + 1)], in_=osb[h][64:65, :]
                    )
                rrow = work.tile([128, 8], F32, tag="rrow", name=f"rrow{qt}{hp}")
                nc.vector.reciprocal(rrow[:], drow[:])
                rec = [
                    work.tile([1, 512], F32, tag=f"rec{h}", name=f"rec{qt}{hp}{h}")
                    for h in range(2)
                ]
                for h in range(2):
                    nc.sync.dma_start(
                        out=rec[h][:], in_=rrow[:, 4 * h:4 * (h + 1)]
                    )
                b_ps = ps_p.tile([128, 512], F32, tag="pp", name=f"bps{qt}{hp}")
                for h in range(2):
                    nc.tensor.matmul(
                        b_ps[h * 64:(h + 1) * 64, :],
                        lhsT=ones64[:],
                        rhs=rec[h][:],
                        start=True,
                        stop=True,
                    )
                outt = fin.tile([128, 512], F32, tag="outt", bufs=1, name=f"outt{qt}{hp}")
                for h in range(2):
                    nc.vector.tensor_tensor(
                        out=outt[h * 64:(h + 1) * 64, :],
                        in0=osb[h][0:64, :],
                        in1=b_ps[h * 64:(h + 1) * 64, :],
                        op=ALU.mult,
                    )
                outf = fin.tile([128, 512], F32, tag="outf", bufs=2, name=f"outf{qt}{hp}")
                nc.vector.tensor_tensor(
                    out=outf[:],
                    in0=outt[:],
                    in1=b3c[:, hp:hp + 1].to_broadcast((128, 512)),
                    op=ALU.add,
                )
                nc.sync.dma_start(out=out_d[hp, :, qt, :], in_=outf[:])

            # ---- phase-batched slots ----
            proj_q_chunk(0)
            proj_q_chunk(1)

            nm_ring = {}
            exp_ring = {}
            MASK_LAG = 2
            o_ps_cur = None
            for slot in range(5):
                gi = GROUPS[slot] if slot < 4 else None
                gp = GROUPS[slot - 1] if slot > 0 else None
                if gp is not None:
                    o_ps_cur = [
                        ps_o.tile([65, 512], F32, tag=f"o{h}", name=f"ops{slot}{h}")
                        for h in range(2)
                    ]
                for kc in range(NKC):
                    # stream projections into the first two slots
                    if slot == 0:
                        if kc % 4 == 0:
                            proj_k_chunk(kc // 4)
                        if kc % 2 == 1:
                            proj_v_chunk(kc // 2)
                    elif slot == 1 and kc % 2 == 1:
                        proj_v_chunk(16 + kc // 2)
                    if gp is not None:
                        av_chunk(gp[1], kc, nm_ring.pop((gp, kc)), o_ps_cur)
                    if gi is not None:
                        s_ps = score_chunk(gi[0], gi[1], kc)
                        exp_ring[(gi, kc)] = exp_chunk(gi[0], gi[1], kc, s_ps)
                        if kc >= MASK_LAG:
                            kcm = kc - MASK_LAG
                            nm_ring[(gi, kcm)] = mask_chunk(
                                gi[0], gi[1], kcm, exp_ring.pop((gi, kcm))
                            )
                if gi is not None:
                    for kcm in range(NKC - MASK_LAG, NKC):
                        nm_ring[(gi, kcm)] = mask_chunk(
                            gi[0], gi[1], kcm, exp_ring.pop((gi, kcm))
                        )
                if gp is not None:
                    attn_tail(gp[0], gp[1], o_ps_cur)

    _split_multi_waits(nc)
    return nc


def _prep_inputs(q, k, v, rns_indices, W1, b1, W2, b2, W3, b3):
    bf = ml_dtypes.bfloat16
    q = np.asarray(q, np.float32)
    k = np.asarray(k, np.float32)
    v = np.asarray(v, np.float32)
    idx = np.asarray(rns_indices)
    W1 = np.asarray(W1, np.float32)
    W2 = np.asarray(W2, np.float32)
    W3 = np.asarray(W3, np.float32)
    b1 = np.asarray(b1, np.float32)
    b2 = np.asarray(b2, np.float32)
    b3 = np.asarray(b3, np.float32)
    scale = 1.0 / np.sqrt(DH)

    def part3(x2d, n):  # (512, n) -> (128, 4, n)
        return np.ascontiguousarray(
            x2d.reshape(4, 128, n).transpose(1, 0, 2)
        ).astype(bf)

    def _aug_w3(W3h):  # (256, 512) -> (128, 4, 260) with zero cols at ones slots
        wt = np.zeros((DIM, 260), np.float32)
        for h in range(4):
            wt[:, h * 65:h * 65 + 64] = W3h[h * 64:(h + 1) * 64, :].T
        return part3(wt, 260)

    masks = []
    for b in range(BS):
        m = np.zeros((NP, MNP), np.float32)
        m[np.arange(NP)[:, None], idx[b]] = 1.0
        mt = m.T.reshape(NKC, 128, NP).transpose(1, 0, 2)
        masks.append(np.ascontiguousarray(mt).astype(bf))

    qkv_t = []
    for b in range(BS):
        qkv_t.append(
            (
                part3(q[:, b, :].T, NP),
                part3(k[:, b, :].T, MNP),
                part3(v[:, b, :].T, MNP),
            )
        )

    in_maps = []
    for core in range(N_CORES):
        b, hg = core // 2, core % 2
        sl = slice(hg * HG_CH, (hg + 1) * HG_CH)
        qtb, ktb, vtb = qkv_t[b]
        im = {
            "qt": qtb,
            "kt": ktb,
            "vt": vtb,
            "w1t": part3(W1[sl, :].T * scale, HG_CH),
            "w2t": part3(W2[sl, :].T, HG_CH),
            "w3t": _aug_w3(W3[sl, :]),
            "b1c": np.ascontiguousarray(
                (b1[sl] * scale).reshape(2, 128).T
            ).astype(np.float32),
            "b2c": np.ascontiguousarray(b2[sl].reshape(2, 128).T).astype(np.float32),
            "b3c": np.ascontiguousarray(b3[sl].reshape(2, 128).T).astype(np.float32),
            "maskt": masks[b],
            "ones64": np.ones((1, 64), np.float32),
        }
        in_maps.append(im)
    return in_maps


def kernel(q, k, v, rns_indices, W1, b1, W2, b2, W3, b3):
    nc = _build_nc()
    in_maps = _prep_inputs(q, k, v, rns_indices, W1, b1, W2, b2, W3, b3)
    res = run_bass_kernel_spmd(
        nc,
        in_maps,
        core_ids=list(range(N_CORES)),
        trace=run_opts["trace"],
        **run_opts["trace_kwargs"],
    )
    _last_results["res"] = res

    out = np.empty((NP, BS, DIM), np.float32)
    for core in range(N_CORES):
        b, hg = core // 2, core % 2
        r = np.asarray(res.results[core]["outt"], np.float32)  # (2,128,2,512)
        arr = r.transpose(2, 3, 0, 1).reshape(NP, HG_CH)
        out[:, b, hg * HG_CH:(hg + 1) * HG_CH] = arr
    return out


# revision 13
# speedup vs baseline: 1.1410x; 1.1410x over previous
"""Sparse cross-attention kernel for Trainium2 (8 NeuronCores, SPMD).

Problem: nn_CrossAttn (NP=1024 queries, MNP=4096 keys, BS=4, DIM=512,
NH=8 heads, dh=64, TOPK=32 sparse mask shared across heads).

Sharding: core = (batch b, head-group hg).  4 batches x 2 head-groups of 4
heads each.  Each core computes its batch's attention for its 4 heads and
writes a (256 ch, 1024 q) transposed output block; the host reassembles.

v2 design (phase-batched, all PE matmuls in 128x128 mode - no tiling-mode
switches):
  - scores use K=128 with zero-padded per-head qhT (qhTz0/qhTz1) so the
    score and AV matmuls share one PE tiling mode and pipeline cleanly.
  - per (qt, hp) "group" of 32 key chunks: slot i emits scores+exp+mask for
    group i interleaved with AV matmuls of group i-1 (numerm ring of 32).
  - exp split: ACT table-exp for 3/4 chunks; DVE Schraudolph bit-trick exp
    (int32 tensor_scalar + bitcast) for kc%4==3.
  - mask multiply split: GpSimd for kc%8 in {1,2,5} (bf16 chunks), DVE else.
  - V projection has no bias matmuls: b3 passes through softmax (weights sum
    to 1) and is added at the tail; 'ones' denominator columns are memset.
"""

import numpy as np
import ml_dtypes

import concourse.bass as bass
import concourse.mybir as mybir
import concourse.tile as tile
from concourse.bass_utils import run_bass_kernel_spmd

BF16 = mybir.dt.bfloat16
F32 = mybir.dt.float32
I32 = mybir.dt.int32
AF = mybir.ActivationFunctionType
ALU = mybir.AluOpType

NH = 8
DIM = 512
NP = 1024
MNP = 4096
BS = 4
DH = 64
N_CORES = 8
HG_CH = 256          # channels per head-group (4 heads x 64)
NKC = MNP // 128     # 32 key chunks

# Schraudolph fast-exp constants (tuned C for scores in [-2.5, 2.5])
SCH_A = float(np.float32(2 ** 23 / np.log(2.0)))
SCH_B = float(np.float32(127 * 2 ** 23 - 366000))

GROUPS = [(0, 0), (0, 1), (1, 0), (1, 1)]  # (qt, hp)


def _is_dve_exp(kc):
    return kc % 4 == 3


def _is_gp_mask(kc):
    return kc % 8 in (1, 2, 5)


# options test.py can flip
run_opts = {"trace": False, "trace_kwargs": {}}
_last_results = {}


def _split_multi_waits(nc):
    """This container's walrus encodes only ONE sync-wait per TPB instruction
    (newer Tile emits several).  Split extras onto preceding NOPs."""
    eng_ok = {
        mybir.EngineType.PE,
        mybir.EngineType.Activation,
        mybir.EngineType.DVE,
        mybir.EngineType.Pool,
        mybir.EngineType.SP,
    }
    for fn in nc.m.functions:
        for blk in fn.blocks:
            insts = blk.instructions
            out = []
            changed = False
            for inst in insts:
                si = inst.sync_info
                if (
                    si is not None
                    and si.on_wait
                    and len(si.on_wait) > 1
                    and inst.engine in eng_ok
                ):
                    waits = list(si.on_wait)
                    for j, w in enumerate(waits[:-1]):
                        out.append(
                            mybir.InstNoOp(
                                name=f"{inst.name}-w{j}",
                                engine=inst.engine,
                                ins=[],
                                outs=[],
                                sync_info=mybir.SyncInfo(on_wait=[w], on_update=[]),
                            )
                        )
                    inst.sync_info = mybir.SyncInfo(
                        on_wait=[waits[-1]], on_update=list(si.on_update)
                    )
                    changed = True
                out.append(inst)
            if changed:
                blk.instructions = out


def _build_nc() -> bass.Bass:
    nc = bass.Bass()

    qt_d = nc.dram_tensor("qt", [128, 4, NP], BF16, kind="ExternalInput")
    kt_d = nc.dram_tensor("kt", [128, 4, MNP], BF16, kind="ExternalInput")
    vt_d = nc.dram_tensor("vt", [128, 4, MNP], BF16, kind="ExternalInput")
    w1t_d = nc.dram_tensor("w1t", [128, 4, HG_CH], BF16, kind="ExternalInput")
    w2t_d = nc.dram_tensor("w2t", [128, 4, HG_CH], BF16, kind="ExternalInput")
    w3t_d = nc.dram_tensor("w3t", [128, 4, 260], BF16, kind="ExternalInput")
    b1c_d = nc.dram_tensor("b1c", [128, 2], F32, kind="ExternalInput")
    b2c_d = nc.dram_tensor("b2c", [128, 2], F32, kind="ExternalInput")
    b3c_d = nc.dram_tensor("b3c", [128, 2], F32, kind="ExternalInput")
    mask_d = nc.dram_tensor("maskt", [128, NKC, NP], BF16, kind="ExternalInput")
    ones64_d = nc.dram_tensor("ones64", [1, 64], F32, kind="ExternalInput")
    out_d = nc.dram_tensor("outt", [2, 128, 2, 512], F32, kind="ExternalOutput")

    with tile.TileContext(nc) as tc:
        with (
            tc.tile_pool(name="const", bufs=1) as const,
            tc.tile_pool(name="big", bufs=1) as big,
            tc.tile_pool(name="kio", bufs=2) as kio,
            tc.tile_pool(name="vio", bufs=2) as vio,
            tc.tile_pool(name="nbp", bufs=2) as nbp,
            tc.tile_pool(name="nip", bufs=1) as nip,
            tc.tile_pool(name="nmp", bufs=32) as nmp,
            tc.tile_pool(name="work", bufs=1) as work,
            tc.tile_pool(name="fin", bufs=2) as fin,
            tc.tile_pool(name="ps_s", bufs=2, space="PSUM") as ps_s,
            tc.tile_pool(name="ps_o", bufs=1, space="PSUM") as ps_o,
            tc.tile_pool(name="ps_p", bufs=2, space="PSUM") as ps_p,
        ):
            # ---- constants / weights ----
            w1t = const.tile([128, 4, HG_CH], BF16, tag="w1t")
            w2t = const.tile([128, 4, HG_CH], BF16, tag="w2t")
            w3t = const.tile([128, 4, 260], BF16, tag="w3t")
            b1c = const.tile([128, 2], F32, tag="b1c")
            b2c = const.tile([128, 2], F32, tag="b2c")
            b3c = const.tile([128, 2], F32, tag="b3c")
            ones64 = const.tile([1, 64], F32, tag="ones64")
            for sb, dr in (
                (w1t, w1t_d), (w2t, w2t_d), (w3t, w3t_d),
                (b1c, b1c_d), (b2c, b2c_d), (b3c, b3c_d), (ones64, ones64_d),
            ):
                nc.sync.dma_start(out=sb[:], in_=dr[:])

            # ---- big resident tensors ----
            # maskt pieces are DMA'd just-in-time inside slot 0 so the q/k
            # projection DMAs are not stuck behind 8MB of mask traffic.
            maskt = big.tile([128, NKC, NP], BF16, tag="maskt")

            def load_mask_piece(i):
                nc.sync.dma_start(
                    out=maskt[:, 4 * i:4 * (i + 1), :],
                    in_=mask_d[:, 4 * i:4 * (i + 1), :],
                )

            qhTz0 = big.tile([128, 2, NP], BF16, tag="qhTz0")
            qhTz1 = big.tile([128, 2, NP], BF16, tag="qhTz1")
            khT = big.tile([128, 2, MNP], BF16, tag="khT")
            vh = big.tile([128, NKC, 260], BF16, tag="vh")

            # zero halves of padded q, ones-columns of vh
            nc.gpsimd.memset(qhTz0[64:128, :, :], 0.0)
            nc.gpsimd.memset(qhTz1[0:64, :, :], 0.0)
            vh_ones = vh[:].rearrange("p k (h c) -> p k h c", h=4)[:, :, :, 64:65]
            nc.gpsimd.memset(vh_ones, 1.0)

            # ---- q projection (zero-padded per head halves) ----
            def proj_q_chunk(nq):
                qts = kio.tile([128, 4, 512], BF16, tag="kts", name=f"qts{nq}")
                nc.sync.dma_start(out=qts[:], in_=qt_d[:, :, nq * 512:(nq + 1) * 512])
                for pair in range(2):
                    pt = ps_p.tile([128, 512], F32, tag="pp", name=f"qp{nq}{pair}")
                    for c in range(4):
                        nc.tensor.matmul(
                            pt[:],
                            lhsT=w1t[:, c, pair * 128:(pair + 1) * 128],
                            rhs=qts[:, c, :],
                            start=(c == 0),
                            stop=(c == 3),
                        )
                    nc.vector.tensor_tensor(
                        out=qhTz0[0:64, pair, nq * 512:(nq + 1) * 512],
                        in0=pt[0:64, :],
                        in1=b1c[0:64, pair:pair + 1].to_broadcast((64, 512)),
                        op=ALU.add,
                    )
                    nc.vector.tensor_tensor(
                        out=qhTz1[64:128, pair, nq * 512:(nq + 1) * 512],
                        in0=pt[64:128, :],
                        in1=b1c[64:128, pair:pair + 1].to_broadcast((64, 512)),
                        op=ALU.add,
                    )

            def proj_k_chunk(nq):
                kts = kio.tile([128, 4, 512], BF16, tag="kts", name=f"kts{nq}")
                nc.sync.dma_start(out=kts[:], in_=kt_d[:, :, nq * 512:(nq + 1) * 512])
                for pair in range(2):
                    pt = ps_p.tile([128, 512], F32, tag="pp", name=f"kp{nq}{pair}")
                    for c in range(4):
                        nc.tensor.matmul(
                            pt[:],
                            lhsT=w2t[:, c, pair * 128:(pair + 1) * 128],
                            rhs=kts[:, c, :],
                            start=(c == 0),
                            stop=(c == 3),
                        )
                    nc.vector.tensor_tensor(
                        out=khT[:, pair, nq * 512:(nq + 1) * 512],
                        in0=pt[:],
                        in1=b2c[:, pair:pair + 1].to_broadcast((128, 512)),
                        op=ALU.add,
                    )

            def proj_v_chunk(kc):
                vts = vio.tile([128, 4, 128], BF16, tag="vts", name=f"vts{kc}")
                nc.sync.dma_start(out=vts[:], in_=vt_d[:, :, kc * 128:(kc + 1) * 128])
                pt = ps_p.tile([128, 260], F32, tag="pp", name=f"vp{kc}")
                for c in range(4):
                    nc.tensor.matmul(
                        pt[:, 0:260],
                        lhsT=vts[:, c, :],
                        rhs=w3t[:, c, :],
                        start=(c == 0),
                        stop=(c == 3),
                    )
                # copy only the 4x64 data columns; ones columns stay memset
                nc.scalar.copy(
                    out=vh[:, kc, :].rearrange("p (h c) -> p h c", h=4)[:, :, 0:64],
                    in_=pt[:, 0:260].rearrange("p (h c) -> p h c", h=4)[:, :, 0:64],
                )

            # ---- attention pieces ----
            def score_chunk(qt, hp, kc):
                s_ps = ps_s.tile([128, 1024], F32, tag="s", name=f"s{qt}{hp}{kc}")
                for h, qz in ((0, qhTz0), (1, qhTz1)):
                    nc.tensor.matmul(
                        s_ps[:, h * 512:(h + 1) * 512],
                        lhsT=khT[:, hp, kc * 128:(kc + 1) * 128],
                        rhs=qz[:, hp, qt * 512:(qt + 1) * 512],
                        start=True,
                        stop=True,
                    )
                return s_ps

            def exp_chunk(qt, hp, kc, s_ps):
                if _is_dve_exp(kc):
                    ni = nip.tile([128, 1024], I32, tag="ni", name=f"ni{qt}{hp}{kc}")
                    nc.vector.tensor_scalar(
                        out=ni[:], in0=s_ps[:],
                        scalar1=SCH_A, scalar2=SCH_B,
                        op0=ALU.mult, op1=ALU.add,
                    )
                    return ni
                nb = nbp.tile([128, 1024], BF16, tag="nb", name=f"nb{qt}{hp}{kc}")
                nc.scalar.activation(nb[:], s_ps[:], AF.Exp)
                return nb

            def mask_chunk(qt, hp, kc, nsrc):
                nm = nmp.tile([128, 1024], BF16, tag="nm", name=f"nm{qt}{hp}{kc}")
                eng = nc.gpsimd if (_is_gp_mask(kc) and not _is_dve_exp(kc)) else nc.vector
                for h in range(2):
                    in0 = nsrc[:, h * 512:(h + 1) * 512]
                    if _is_dve_exp(kc):
                        in0 = in0.bitcast(F32)
                    eng.tensor_tensor(
                        out=nm[:, h * 512:(h + 1) * 512],
                        in0=in0,
                        in1=maskt[:, kc, qt * 512:(qt + 1) * 512],
                        op=ALU.mult,
                    )
                return nm

            def av_chunk(hp, kc, nm, o_ps):
                for h in range(2):
                    ch = (2 * hp + h) * 65
                    nc.tensor.matmul(
                        o_ps[h][:],
                        lhsT=vh[:, kc, ch:ch + 65],
                        rhs=nm[:, h * 512:(h + 1) * 512],
                        start=(kc == 0),
                        stop=(kc == NKC - 1),
                    )

            def attn_tail(qt, hp, o_ps):
                # free o_ps quickly: copy [65,512] (row 64 = denominator)
                osb = [
                    fin.tile([65, 512], F32, tag=f"osb{h}", bufs=1, name=f"osb{qt}{hp}{h}")
                    for h in range(2)
                ]
                for h in range(2):
                    nc.scalar.copy(out=osb[h][:], in_=o_ps[h][:])
                # transpose den rows to partitions via DMA, reciprocal, back
                drow = work.tile([128, 8], F32, tag="drow", name=f"drow{qt}{hp}")
                for h in range(2):
                    nc.sync.dma_start(
                        out=drow[:, 4 * h:4 * (h + 1)], in_=osb[h][64:65, :]
                    )
                rrow = work.tile([128, 8], F32, tag="rrow", name=f"rrow{qt}{hp}")
                nc.vector.reciprocal(rrow[:], drow[:])
                rec = [
                    work.tile([1, 512], F32, tag=f"rec{h}", name=f"rec{qt}{hp}{h}")
                    for h in range(2)
                ]
                for h in range(2):
                    nc.sync.dma_start(
                        out=rec[h][:], in_=rrow[:, 4 * h:4 * (h + 1)]
                    )
                b_ps = ps_p.tile([128, 512], F32, tag="pp", name=f"bps{qt}{hp}")
                for h in range(2):
                    nc.tensor.matmul(
                        b_ps[h * 64:(h + 1) * 64, :],
                        lhsT=ones64[:],
                        rhs=rec[h][:],
                        start=True,
                        stop=True,
                    )
                outt = fin.tile([128, 512], F32, tag="outt", bufs=1, name=f"outt{qt}{hp}")
                for h in range(2):
                    nc.vector.tensor_tensor(
                        out=outt[h * 64:(h + 1) * 64, :],
                        in0=osb[h][0:64, :],
                        in1=b_ps[h * 64:(h + 1) * 64, :],
                        op=ALU.mult,
                    )
                outf = fin.tile([128, 512], F32, tag="outf", bufs=2, name=f"outf{qt}{hp}")
                nc.vector.tensor_tensor(
                    out=outf[:],
                    in0=outt[:],
                    in1=b3c[:, hp:hp + 1].to_broadcast((128, 512)),
                    op=ALU.add,
                )
                nc.sync.dma_start(out=out_d[hp, :, qt, :], in_=outf[:])

            # ---- phase-batched slots ----
            proj_q_chunk(0)
            proj_q_chunk(1)

            nm_ring = {}
            exp_ring = {}
            MASK_LAG = 2
            o_ps_cur = None
            for slot in range(5):
                gi = GROUPS[slot] if slot < 4 else None
                gp = GROUPS[slot - 1] if slot > 0 else None
                if gp is not None:
                    o_ps_cur = [
                        ps_o.tile([65, 512], F32, tag=f"o{h}", name=f"ops{slot}{h}")
                        for h in range(2)
                    ]
                for kc in range(NKC):
                    # stream projections into the first two slots
                    if slot == 0:
                        if kc % 4 == 0:
                            proj_k_chunk(kc // 4)
                            load_mask_piece(kc // 4)
                        if kc % 2 == 1:
                            proj_v_chunk(kc // 2)
                    elif slot == 1 and kc % 2 == 1:
                        proj_v_chunk(16 + kc // 2)
                    if gp is not None:
                        av_chunk(gp[1], kc, nm_ring.pop((gp, kc)), o_ps_cur)
                    if gi is not None:
                        s_ps = score_chunk(gi[0], gi[1], kc)
                        exp_ring[(gi, kc)] = exp_chunk(gi[0], gi[1], kc, s_ps)
                        if kc >= MASK_LAG:
                            kcm = kc - MASK_LAG
                            nm_ring[(gi, kcm)] = mask_chunk(
                                gi[0], gi[1], kcm, exp_ring.pop((gi, kcm))
                            )
                if gi is not None:
                    for kcm in range(NKC - MASK_LAG, NKC):
                        nm_ring[(gi, kcm)] = mask_chunk(
                            gi[0], gi[1], kcm, exp_ring.pop((gi, kcm))
                        )
                if gp is not None:
                    attn_tail(gp[0], gp[1], o_ps_cur)

    _split_multi_waits(nc)
    return nc


def _prep_inputs(q, k, v, rns_indices, W1, b1, W2, b2, W3, b3):
    bf = ml_dtypes.bfloat16
    q = np.asarray(q, np.float32)
    k = np.asarray(k, np.float32)
    v = np.asarray(v, np.float32)
    idx = np.asarray(rns_indices)
    W1 = np.asarray(W1, np.float32)
    W2 = np.asarray(W2, np.float32)
    W3 = np.asarray(W3, np.float32)
    b1 = np.asarray(b1, np.float32)
    b2 = np.asarray(b2, np.float32)
    b3 = np.asarray(b3, np.float32)
    scale = 1.0 / np.sqrt(DH)

    def part3(x2d, n):  # (512, n) -> (128, 4, n)
        return np.ascontiguousarray(
            x2d.reshape(4, 128, n).transpose(1, 0, 2)
        ).astype(bf)

    def _aug_w3(W3h):  # (256, 512) -> (128, 4, 260) with zero cols at ones slots
        wt = np.zeros((DIM, 260), np.float32)
        for h in range(4):
            wt[:, h * 65:h * 65 + 64] = W3h[h * 64:(h + 1) * 64, :].T
        return part3(wt, 260)

    masks = []
    for b in range(BS):
        m = np.zeros((NP, MNP), np.float32)
        m[np.arange(NP)[:, None], idx[b]] = 1.0
        mt = m.T.reshape(NKC, 128, NP).transpose(1, 0, 2)
        masks.append(np.ascontiguousarray(mt).astype(bf))

    qkv_t = []
    for b in range(BS):
        qkv_t.append(
            (
                part3(q[:, b, :].T, NP),
                part3(k[:, b, :].T, MNP),
                part3(v[:, b, :].T, MNP),
            )
        )

    in_maps = []
    for core in range(N_CORES):
        b, hg = core // 2, core % 2
        sl = slice(hg * HG_CH, (hg + 1) * HG_CH)
        qtb, ktb, vtb = qkv_t[b]
        im = {
            "qt": qtb,
            "kt": ktb,
            "vt": vtb,
            "w1t": part3(W1[sl, :].T * scale, HG_CH),
            "w2t": part3(W2[sl, :].T, HG_CH),
            "w3t": _aug_w3(W3[sl, :]),
            "b1c": np.ascontiguousarray(
                (b1[sl] * scale).reshape(2, 128).T
            ).astype(np.float32),
            "b2c": np.ascontiguousarray(b2[sl].reshape(2, 128).T).astype(np.float32),
            "b3c": np.ascontiguousarray(b3[sl].reshape(2, 128).T).astype(np.float32),
            "maskt": masks[b],
            "ones64": np.ones((1, 64), np.float32),
        }
        in_maps.append(im)
    return in_maps


def kernel(q, k, v, rns_indices, W1, b1, W2, b2, W3, b3):
    nc = _build_nc()
    in_maps = _prep_inputs(q, k, v, rns_indices, W1, b1, W2, b2, W3, b3)
    res = run_bass_kernel_spmd(
        nc,
        in_maps,
        core_ids=list(range(N_CORES)),
        trace=run_opts["trace"],
        **run_opts["trace_kwargs"],
    )
    _last_results["res"] = res

    out = np.empty((NP, BS, DIM), np.float32)
    for core in range(N_CORES):
        b, hg = core // 2, core % 2
        r = np.asarray(res.results[core]["outt"], np.float32)  # (2,128,2,512)
        arr = r.transpose(2, 3, 0, 1).reshape(NP, HG_CH)
        out[:, b, hg * HG_CH:(hg + 1) * HG_CH] = arr
    return out
